# revision 28
# baseline (speedup 1.0000x reference)
"""Multi-head attention kernel for Trainium2, data-parallel over 8 NeuronCores.

Problem: B=16, N=1024, D=768, H=12 heads (hd=64), fp32 I/O.
  qkv = x @ w_qkv + b_qkv ; attention ; out = attn_out @ w_proj + b_proj

Sharding: batch data-parallel — core c handles batches [2c, 2c+2); weights
replicated. Inside each core, the two batches are processed sequentially.

Layout strategy (bf16 matmuls at 1 cyc/row; PE held continuously busy so it
ramps to the 2.4 GHz p-state):
  - host pre-transposes x to xT [768, T] so the in-feature contraction has
    features on partitions for both operands.
  - Q^T, K^T computed feature-major [768, N]: lhsT = w_qkv cols, rhs = xT.
    A 128-row feature tile holds a PAIR of heads (2x64).
  - V computed token-major [N, 768], stored bf16 with a ones column appended
    per head (v_ext [128, 65]).
  - scores^T tiles [128 j, 512 q] per head -> one ACT exp op [128, 1024]
    covers both heads of a pair (softmax scale folded into exp's scale).
  - U^T = sum_j exp * v_ext accumulates in PSUM [65, 512]; row 64 is the
    softmax denominator (ones column).
  - normalization: reciprocal_approx_fast on the two [1,512] denominator
    rows -> dinv2 [2,512]; gpsimd broadcast into rb [128,512]; one
    tensor_mul per head reads the U rows STRAIGHT FROM PSUM (inputs
    partition-aligned at 0..63, write shifted for head 1) and writes the
    normalized bf16 uT — no SBUF staging copy.
  - proj: lhsT = uT tile, rhs = w_proj; + b_proj via broadcast add.
    Output lands token-major [T, 768] == final layout.
  - scores are emitted one key-tile ahead of attn@V so exp latency is
    hidden; QKV of the next batch / proj of the previous batch are
    interleaved into the attention blocks as TensorE filler.

Schedule refinements vs the first working version:
  - bias DMAs are emitted FIRST (running-threshold waits: the old order made
    the first evacuations wait for the whole x/wqkv stream, pacing the PE at
    startup ~14us).
  - wqkv streams in per-k column-group order (QK pair-0/1 cols + V cols per
    k chunk, the rest later), and the prologue runs 4 QKV units
    k-chunk-interleaved across 4 PSUM accumulators (2 psQ bufs + 2 halves of
    a psS tile), so the PE consumes each x chunk as it lands instead of one
    unit serially riding the DMA.
  - batch-1 attention runs ih-major so its ih=0 projection tiles become
    TensorE filler mid-attention instead of a serial tail; the final norm
    chain is covered by reserved filler.
"""

import contextlib
import ctypes
import os
import sys
import types

import numpy as np

# ---------------------------------------------------------------------------
# NTFF profiling shim: bass_utils's trace path imports
# antenv.axon_hooks.get_axon_ntff_profile_hook, which this container's antenv
# lacks. Register a ctypes-based equivalent so BASS_TRACE=1 works. Harmless
# if tracing is never requested.
# ---------------------------------------------------------------------------


def _install_ntff_shim():
    if "antenv.axon_hooks" in sys.modules:
        return
    so_path = "/opt/axon/libaxon_pjrt.so"
    hook = None
    try:
        lib = ctypes.CDLL(so_path)
        if hasattr(lib, "axon_start_nrt_profile"):
            lib.axon_start_nrt_profile.argtypes = [
                ctypes.POINTER(ctypes.c_int64),
                ctypes.c_size_t,
            ]
            lib.axon_start_nrt_profile.restype = ctypes.c_int64
            lib.axon_stop_nrt_profile.argtypes = [ctypes.c_char_p]
            lib.axon_stop_nrt_profile.restype = ctypes.c_int64

            @contextlib.contextmanager
            def _hook(output_dir, device_ids):
                import jax

                jax.devices()
                if device_ids:
                    ids = (ctypes.c_int64 * len(device_ids))(*device_ids)
                    rc = lib.axon_start_nrt_profile(ids, len(device_ids))
                else:
                    rc = lib.axon_start_nrt_profile(None, 0)
                if rc != 0:
                    raise RuntimeError(f"axon_start_nrt_profile rc={rc}")
                try:
                    yield
                finally:
                    n = lib.axon_stop_nrt_profile(str(output_dir).encode())
                    print(f"ntff profile: {n} file(s) in {output_dir}",
                          file=sys.stderr)

            hook = _hook
    except OSError:
        pass
    mod = types.ModuleType("antenv.axon_hooks")
    mod.get_axon_ntff_profile_hook = lambda: hook
    mod.set_axon_ntff_profile_hook = lambda h: None
    sys.modules["antenv.axon_hooks"] = mod


_install_ntff_shim()

import concourse.bacc as bacc  # noqa: E402
import concourse.mybir as mybir  # noqa: E402
import concourse.tile as tile  # noqa: E402
from concourse.bass_utils import run_bass_kernel_spmd  # noqa: E402

F32 = mybir.dt.float32
F32R = mybir.dt.float32r
BF16 = mybir.dt.bfloat16
AF = mybir.ActivationFunctionType

# Problem constants (per core)
NB = 2        # batches per core
TN = 1024     # tokens per batch
T = NB * TN   # tokens per core
D = 768
H = 12
HD = 64
D3 = 3 * D
KT = D // 128          # 6 contraction tiles
NPAIR = H // 2         # 6 head pairs
NJT = TN // 128        # 8 key tiles per batch
SCALE = HD ** -0.5


def build():
    nc = bacc.Bacc(None)
    xT_d = nc.declare_dram_parameter("xT", [D, T], BF16, isOutput=False)
    wqkv_d = nc.declare_dram_parameter("wqkv", [D, D3], BF16, isOutput=False)
    wproj_d = nc.declare_dram_parameter("wproj", [D, D], BF16, isOutput=False)
    bqk_d = nc.declare_dram_parameter("bqk", [128, 12], F32, isOutput=False)
    bv_d = nc.declare_dram_parameter("bv", [1, D], BF16, isOutput=False)
    bproj_d = nc.declare_dram_parameter("bproj", [1, D], BF16, isOutput=False)
    out_d = nc.declare_dram_parameter("out", [T, D], F32, isOutput=True)

    with tile.TileContext(nc) as tc:
        with (
            nc.allow_low_precision(reason="bf16 attention pipeline"),
            tc.tile_pool(name="const", bufs=1) as cpool,
            tc.tile_pool(name="xu", bufs=2) as xupool,
            tc.tile_pool(name="qk", bufs=2) as qkpool,
            tc.tile_pool(name="vsb", bufs=2) as vpool,
            tc.tile_pool(name="esb", bufs=5) as epool,
            tc.tile_pool(name="stg", bufs=3) as spool,
            tc.tile_pool(name="dnv", bufs=3) as dpool,
            tc.tile_pool(name="rbp", bufs=4) as rbpool,
            tc.tile_pool(name="osb", bufs=3) as opool,
            tc.tile_pool(name="psS", bufs=2, space="PSUM") as psS,
            tc.tile_pool(name="psU", bufs=2, space="PSUM") as psU,
            tc.tile_pool(name="psQ", bufs=2, space="PSUM") as psQ,
        ):
            def dma_striped(dst, src, stripes):
                # split one logical transfer into partition stripes — each
                # dma_start lands on its own hardware queue, so this divides
                # the per-queue descriptor serialization
                step = 128 // stripes
                for s in range(stripes):
                    nc.sync.dma_start(
                        dst[s * step : (s + 1) * step], src[s * step : (s + 1) * step]
                    )

            wqkv = cpool.tile([128, KT, D3], BF16, tag="wqkv")
            wqkv_src = wqkv_d.ap().rearrange("(ko p) n -> p ko n", p=128)

            # --- per-batch contexts -------------------------------------
            class Batch:
                pass

            bats = []
            for b in range(NB):
                B_ = Batch()
                B_.tok0 = b * TN
                B_.qT = qkpool.tile([128, NPAIR, TN], BF16, tag="qT",
                                    name=f"qT{b}")
                B_.kT = qkpool.tile([128, NPAIR, TN], BF16, tag="kT",
                                    name=f"kT{b}")
                B_.vsb = vpool.tile([128, NJT, H, HD + 1], BF16, tag="v",
                                    name=f"v{b}")
                B_.uT = xupool.tile([128, KT, TN], BF16, tag="u",
                                    name=f"uT{b}")
                bats.append(B_)
            # both batches share one x tile: full 2048-token rows give 4KB
            # DMA runs (half the descriptor count of per-batch chunks)
            xcore = xupool.tile([128, KT, T], BF16, tag="x", bufs=1)

            xT_src = xT_d.ap().rearrange("(ko p) n -> p ko n", p=128)

            def wq_cols(k, c0, c1):
                nc.sync.dma_start(
                    wqkv[:, k : k + 1, c0:c1], wqkv_src[:, k : k + 1, c0:c1]
                )

            def emit_qk_unit(B_, m, ih):
                # one [128 feat, 512 tok] Q^T or K^T tile (m<6: Q, else K)
                dst = B_.qT if m < 6 else B_.kT
                hp = m % 6
                ps = psQ.tile([128, 512], F32, tag="ps")
                i0 = B_.tok0 + ih * 512
                for k in range(KT):
                    nc.tensor.matmul(
                        ps[:],
                        wqkv[:, k, m * 128 : (m + 1) * 128],
                        xcore[:, k, i0 : i0 + 512],
                        start=(k == 0),
                        stop=(k == KT - 1),
                    )
                nc.vector.tensor_scalar_add(
                    dst[:, hp, ih * 512 : (ih + 1) * 512],
                    ps[:],
                    bqk[:, m : m + 1],
                )

            def emit_v_unit(B_, t, nh):
                # one [128 tok, 384 feat] V tile into the v_ext slots
                ps = psQ.tile([128, 384], F32, tag="ps")
                t0_ = B_.tok0 + t * 128
                for k in range(KT):
                    nc.tensor.matmul(
                        ps[:],
                        xcore[:, k, t0_ : t0_ + 128],
                        wqkv[:, k, 2 * D + nh * 384 : 2 * D + (nh + 1) * 384],
                        start=(k == 0),
                        stop=(k == KT - 1),
                    )
                nc.vector.tensor_add(
                    B_.vsb[:, t, nh * 6 : (nh + 1) * 6, 0:HD],
                    ps[:],
                    bvb[:, nh * 384 : (nh + 1) * 384],
                )

            def emit_qk_wave(B_, units):
                """Up to 4 QK units accumulated k-chunk-interleaved so the PE
                consumes each x/wqkv chunk as its DMA lands (the serial unit
                order stalls ~2.6us per chunk at startup). Accumulators: the
                two psQ bufs plus both halves of one psS tile (idle during
                the prologue)."""
                accs = []
                wide = psS.tile([128, 1024], F32, tag="s")
                for i in range(len(units)):
                    if i < 2:
                        accs.append(
                            psQ.tile([128, 512], F32, tag="ps",
                                     name=f"pwq{i}")[:]
                        )
                    else:
                        accs.append(wide[:, (i - 2) * 512 : (i - 1) * 512])
                for k in range(KT):
                    for acc, (m, ih) in zip(accs, units):
                        i0 = B_.tok0 + ih * 512
                        nc.tensor.matmul(
                            acc,
                            wqkv[:, k, m * 128 : (m + 1) * 128],
                            xcore[:, k, i0 : i0 + 512],
                            start=(k == 0),
                            stop=(k == KT - 1),
                        )
                for acc, (m, ih) in zip(accs, units):
                    dst = B_.qT if m < 6 else B_.kT
                    nc.vector.tensor_scalar_add(
                        dst[:, m % 6, ih * 512 : (ih + 1) * 512],
                        acc,
                        bqk[:, m : m + 1],
                    )

            def emit_v_wave(B_, ts, nh):
                """Up to 4 V units, k-chunk-interleaved (see emit_qk_wave)."""
                accs = []
                wide = psS.tile([128, 1024], F32, tag="s")
                for i in range(len(ts)):
                    if i < 2:
                        accs.append(
                            psQ.tile([128, 384], F32, tag="ps",
                                     name=f"pwv{i}")[:]
                        )
                    else:
                        accs.append(wide[:, (i - 2) * 512 : (i - 2) * 512 + 384])
                for k in range(KT):
                    for acc, t in zip(accs, ts):
                        t0_ = B_.tok0 + t * 128
                        nc.tensor.matmul(
                            acc,
                            xcore[:, k, t0_ : t0_ + 128],
                            wqkv[:, k, 2 * D + nh * 384 : 2 * D + (nh + 1) * 384],
                            start=(k == 0),
                            stop=(k == KT - 1),
                        )
                for acc, t in zip(accs, ts):
                    nc.vector.tensor_add(
                        B_.vsb[:, t, nh * 6 : (nh + 1) * 6, 0:HD],
                        acc,
                        bvb[:, nh * 384 : (nh + 1) * 384],
                    )

            def emit_proj_unit(B_, t, nh):
                ps = psQ.tile([128, 384], F32, tag="ps")
                for k in range(KT):
                    nc.tensor.matmul(
                        ps[:],
                        B_.uT[:, k, t * 128 : (t + 1) * 128],
                        wproj[:, k, nh * 384 : (nh + 1) * 384],
                        start=(k == 0),
                        stop=(k == KT - 1),
                    )
                ot = opool.tile([128, 384], F32, tag="o")
                nc.vector.tensor_add(
                    ot[:], ps[:], bprojb[:, nh * 384 : (nh + 1) * 384]
                )
                nc.sync.dma_start(
                    out_d.ap()[
                        B_.tok0 + t * 128 : B_.tok0 + (t + 1) * 128,
                        nh * 384 : (nh + 1) * 384,
                    ],
                    ot[:],
                )

            # ---- filler queue: TensorE work interleaved into attention ---
            fill_queue = []

            def pop_fill(n, reserve=0):
                for _ in range(n):
                    if len(fill_queue) > reserve:
                        fill_queue.pop(0)()

            def emit_norm(pending):
                """Broadcast 1/denom (GpSimd, otherwise idle; full-tile dst
                from partition 0 — the only pattern the library op supports)
                and write the normalized pair block into uT. Reads the
                SBUF-staged U (staging decouples the psU buffer recycling:
                consecutive blocks share psU slots, and a norm reading PSUM
                directly stalls the next block's first attnv ~0.6us)."""
                B_, hp, ih, ust, dinv2 = pending
                for h in range(2):
                    rb = rbpool.tile([128, 512], F32, tag="rb",
                                     name=f"rb{h}")
                    nc.gpsimd.partition_broadcast(
                        rb[:], dinv2[0:1, h * 512 : (h + 1) * 512]
                    )
                    nc.vector.tensor_mul(
                        B_.uT[
                            h * 64 : (h + 1) * 64,
                            hp,
                            ih * 512 : (ih + 1) * 512,
                        ],
                        ust[h * 64 : (h + 1) * 64, :],
                        rb[h * 64 : (h + 1) * 64, :],
                    )

            def emit_attn_block(B_, hp, ih, pending, reserve=0,
                                pops=(0, 2, 4, 6)):
                """One (head-pair, query-half) attention block: 8 key tiles of
                scores+exp+attnV, scores one jt ahead of attnV. Fillers pop
                BEFORE the attnv of that jt so a V fill popped at jt is
                emitted ahead of the attnv that consumes it."""
                i0 = ih * 512
                pu = [
                    psU.tile([HD + 1, 512], F32, tag="pu", name=f"pu{h}")
                    for h in range(2)
                ]
                prev_e = None

                def attnv(e, jt):
                    for h in range(2):
                        nc.tensor.matmul(
                            pu[h][:],
                            B_.vsb[:, jt, 2 * hp + h, :],
                            e[:, h * 512 : (h + 1) * 512],
                            start=(jt == 0),
                            stop=(jt == NJT - 1),
                        )

                for jt in range(NJT):
                    ps = psS.tile([128, 1024], F32, tag="s")
                    for h in range(2):
                        nc.tensor.matmul(
                            ps[:, h * 512 : (h + 1) * 512],
                            B_.kT[
                                h * 64 : (h + 1) * 64,
                                hp,
                                jt * 128 : (jt + 1) * 128,
                            ],
                            B_.qT[h * 64 : (h + 1) * 64, hp, i0 : i0 + 512],
                        )
                    e = epool.tile([128, 1024], BF16, tag="e")
                    nc.scalar.activation(e[:], ps[:], AF.Exp, scale=SCALE)
                    if prev_e is not None:
                        attnv(prev_e, jt - 1)
                    if jt == 1 and pending is not None:
                        # the deferred norm goes AFTER the jt==0 filler pop:
                        # its muls wait the cross-engine recip->broadcast
                        # chain, and DVE runs in order — emitted at jt==0
                        # they delay the fill's PSUM evacuation and stall
                        # the PE ~1.4us per block
                        emit_norm(pending)
                        pending = None
                    if jt in pops:
                        pop_fill(1, reserve)
                    prev_e = e
                attnv(prev_e, NJT - 1)
                if pending is not None:
                    emit_norm(pending)

                # evacuate: denominator rows to SBUF staging, then one
                # reciprocal over both; data rows -> ust staging. pu slots
                # release after the copies.
                dtmp = dpool.tile([1, 1024], F32, tag="dtmp", bufs=2)
                dinv2 = dpool.tile([1, 1024], F32, tag="dinv", bufs=2)
                ust = spool.tile([128, 512], F32, tag="ust")
                # denominator extraction + reciprocal first: they head the
                # critical chain (recip -> broadcast -> normalize multiply)
                for h in range(2):
                    nc.vector.tensor_copy(
                        dtmp[0:1, h * 512 : (h + 1) * 512],
                        pu[h][HD : HD + 1, :],
                    )
                nc.vector.reciprocal_approx_fast(dinv2[0:1, :], dtmp[0:1, :])
                for h in range(2):
                    nc.vector.tensor_copy(
                        ust[h * 64 : (h + 1) * 64, :], pu[h][0:HD, :]
                    )
                return (B_, hp, ih, ust, dinv2)

            # ================= global schedule =================
            b0, b1 = bats
            # biases FIRST: DMA waits are running-threshold (waiting on
            # transfer N waits on all earlier ones), and the first QK/V
            # evacuations need these — emitted late they pace the whole
            # prologue behind the x/wqkv bulk.
            bqk = cpool.tile([128, 12], F32, tag="bqk")
            nc.sync.dma_start(bqk[:], bqk_d.ap())
            bv1 = cpool.tile([1, D], BF16, tag="bv1")
            nc.sync.dma_start(bv1[:], bv_d.ap())
            bproj1 = cpool.tile([1, D], BF16, tag="bproj1")
            nc.sync.dma_start(bproj1[:], bproj_d.ap())

            # b0 x + wqkv interleaved per k chunk. wqkv moves as FULL k rows:
            # column-group slices would give 256B DMA runs (4x descriptor
            # overhead, measured to starve mid-attention fills) vs the 4.6KB
            # full-row runs.
            for k in range(KT):
                dma_striped(
                    xcore[:, k : k + 1, 0:TN], xT_src[:, k : k + 1, 0:TN], 2
                )
                dma_striped(wqkv[:, k : k + 1, :], wqkv_src[:, k : k + 1, :], 2)

            # ones column of v_ext via memset — as a DMA this fragments
            # into ~12k 2-byte descriptors and stalls startup ~20us
            nc.vector.memset(b0.vsb[:, :, :, HD : HD + 1], 1.0)
            nc.vector.memset(b1.vsb[:, :, :, HD : HD + 1], 1.0)
            bvb = cpool.tile([128, D], BF16, tag="bvb")
            nc.gpsimd.partition_broadcast(bvb[:], bv1[:])
            bprojb = cpool.tile([128, D], BF16, tag="bprojb")
            nc.gpsimd.partition_broadcast(bprojb[:], bproj1[:])

            # b0 prologue, k-chunk-interleaved 4-unit waves: just Q/K pairs
            # 0,1 — attention starts right after; V nh=0 joins as the first
            # fills, consumed one key-tile ahead of block 0's attnv.
            emit_qk_wave(b0, [(0, 0), (6, 0), (0, 1), (6, 1)])
            emit_qk_wave(b0, [(1, 0), (7, 0), (1, 1), (7, 1)])

            # b1's x, then wproj — in fill consumption order.
            for k in range(KT):
                dma_striped(
                    xcore[:, k : k + 1, TN:T], xT_src[:, k : k + 1, TN:T], 2
                )
            wproj = cpool.tile([128, KT, D], BF16, tag="wproj")
            wproj_src = wproj_d.ap().rearrange("(ko p) n -> p ko n", p=128)
            for k in range(KT):
                nc.sync.dma_start(
                    wproj[:, k : k + 1, :], wproj_src[:, k : k + 1, :]
                )

            # filler during b0 attention, ordered by consumption deadline:
            # V nh=0 tile t by block 0's attnv(t), qk(hp) by block 2*hp,
            # V nh=1 by block 6 (pair 3).
            for t in range(NJT):
                fill_queue.append(lambda t=t: emit_v_unit(b0, t, 0))
            for hp in (2, 3):
                for m in (hp, hp + 6):
                    for ih in range(2):
                        fill_queue.append(
                            lambda m=m, ih=ih: emit_qk_unit(b0, m, ih)
                        )
            for t in range(NJT):
                fill_queue.append(
                    lambda t=t: emit_v_unit(b0, t, 1)
                )
            for hp in (4, 5):
                for m in (hp, hp + 6):
                    for ih in range(2):
                        fill_queue.append(
                            lambda m=m, ih=ih: emit_qk_unit(b0, m, ih)
                        )
            for m in (0, 6, 1, 7):
                for ih in range(2):
                    fill_queue.append(
                        lambda m=m, ih=ih: emit_qk_unit(b1, m, ih)
                    )
            for t in range(NJT):
                fill_queue.append(lambda t=t: emit_v_unit(b1, t, 0))

            pending = None
            for hp in range(NPAIR):
                for ih in range(2):
                    # block 0 pops every jt: it consumes the 8 V nh=0 fills
                    # just ahead of its own attnv
                    pops = tuple(range(NJT)) if (hp, ih) == (0, 0) else (0, 2, 4, 6)
                    pending = emit_attn_block(b0, hp, ih, pending, pops=pops)

            # drain b0 leftovers, then queue b1's remaining QKV (deadline
            # order; pair hp is needed by block 2*hp) and b0's proj as
            # filler during b1 attention
            while fill_queue:
                fill_queue.pop(0)()
            for hp in (2, 3):
                for m in (hp, hp + 6):
                    for ih in range(2):
                        fill_queue.append(
                            lambda m=m, ih=ih: emit_qk_unit(b1, m, ih)
                        )
            for t in range(NJT):
                fill_queue.append(lambda t=t: emit_v_unit(b1, t, 1))
            for hp in (4, 5):
                for m in (hp, hp + 6):
                    for ih in range(2):
                        fill_queue.append(
                            lambda m=m, ih=ih: emit_qk_unit(b1, m, ih)
                        )
            for t in range(NJT):
                for nh in range(2):
                    fill_queue.append(
                        lambda t=t, nh=nh: emit_proj_unit(b0, t, nh)
                    )

            # b1 attention (hp-major, like b0). Before the LAST block, b1's
            # ih=0 projection tiles (t 0-3, complete once the (5,0) norm
            # runs at that block's jt==0) join the filler; reserve holds a
            # few back to cover the final norm chain.
            for hp in range(NPAIR):
                for ih in range(2):
                    last = (hp, ih) == (NPAIR - 1, 1)
                    if last:
                        for t in range(NJT // 2):
                            for nh in range(2):
                                fill_queue.append(
                                    lambda t=t, nh=nh: emit_proj_unit(
                                        b1, t, nh
                                    )
                                )
                    # the last block's first pop waits until jt==1, after
                    # the (5,0) norm lands the uT columns its b1-proj
                    # fills read
                    pending = emit_attn_block(
                        b1, hp, ih, pending,
                        reserve=5 if last else 0,
                        pops=(1, 2, 4, 6) if last else (0, 2, 4, 6),
                    )
            # drain the reserved filler BEFORE the final norm: the drained
            # proj tiles only read ih=0 uT columns, but Tile's tile-granular
            # dependency would chain them behind the final norm if emitted
            # after it (measured 5.3us PE gap). Then the final norm and the
            # ih=1 proj tiles that genuinely need it.
            while fill_queue:
                fill_queue.pop(0)()
            emit_norm(pending)
            for t in range(NJT // 2, NJT):
                for nh in range(2):
                    emit_proj_unit(b1, t, nh)

    nc.compile()
    return nc


_NC_CACHE = None


def _get_nc():
    global _NC_CACHE
    if _NC_CACHE is None:
        _NC_CACHE = build()
    return _NC_CACHE


def _prep_core_inputs(x_c, w_qkv, b_qkv, w_proj, b_proj):
    """Host-side layout prep for one core. x_c: [2, 1024, 768]."""
    xT = np.ascontiguousarray(x_c.reshape(T, D).T).astype(np.float32)
    bqk = np.ascontiguousarray(b_qkv[: 12 * 128].reshape(12, 128).T)
    import ml_dtypes

    bf = ml_dtypes.bfloat16
    return {
        "xT": np.ascontiguousarray(xT.astype(bf)),
        "wqkv": np.ascontiguousarray(w_qkv.astype(bf)),
        "wproj": np.ascontiguousarray(w_proj.astype(bf)),
        "bqk": bqk.astype(np.float32),
        "bv": np.ascontiguousarray(b_qkv[2 * D :].reshape(1, D).astype(bf)),
        "bproj": np.ascontiguousarray(b_proj.reshape(1, D).astype(bf)),
    }


def kernel(x, w_qkv, b_qkv, w_proj, b_proj):
    x = np.asarray(x, dtype=np.float32)
    w_qkv = np.asarray(w_qkv, dtype=np.float32)
    b_qkv = np.asarray(b_qkv, dtype=np.float32)
    w_proj = np.asarray(w_proj, dtype=np.float32)
    b_proj = np.asarray(b_proj, dtype=np.float32)
    B, N, Dd = x.shape
    assert (B, N, Dd) == (16, 1024, 768)

    nc = _get_nc()
    in_maps = [
        _prep_core_inputs(x[2 * c : 2 * c + 2], w_qkv, b_qkv, w_proj, b_proj)
        for c in range(8)
    ]
    res = run_bass_kernel_spmd(nc, in_maps, core_ids=list(range(8)))
    out = np.empty((B, N, Dd), dtype=np.float32)
    for c in range(8):
        out[2 * c : 2 * c + 2] = res.results[c]["out"].reshape(2, N, Dd)
    kernel.last_results = res
    return out


# revision 42
# speedup vs baseline: 1.2154x; 1.2154x over previous
"""Multi-head attention kernel for Trainium2, data-parallel over 8 NeuronCores.

Problem: B=16, N=1024, D=768, H=12 heads (hd=64), fp32 I/O.
  qkv = x @ w_qkv + b_qkv ; attention ; out = attn_out @ w_proj + b_proj

Sharding: batch data-parallel — core c handles batches [2c, 2c+2); weights
replicated. Inside each core, the two batches are processed sequentially.

Layout strategy (bf16 matmuls at 1 cyc/row; PE held continuously busy so it
ramps to the 2.4 GHz p-state):
  - host pre-transposes x to xT [768, T] so the in-feature contraction has
    features on partitions for both operands.
  - Q^T, K^T computed feature-major [768, N]: lhsT = w_qkv cols, rhs = xT.
    A 128-row feature tile holds a PAIR of heads (2x64).
  - V computed token-major [N, 768], stored bf16 with a ones column appended
    per head (v_ext [128, 65]).
  - scores^T tiles [128 j, 512 q] per head -> one ACT exp op [128, 1024]
    covers both heads of a pair (softmax scale folded into exp's scale).
  - U^T = sum_j exp * v_ext accumulates in PSUM [65, 512]; row 64 is the
    softmax denominator (ones column).
  - normalization: the U data + denominator rows are staged to SBUF at block
    end (releasing the PSUM accumulators — consecutive blocks share psU
    slots, so a norm reading PSUM directly stalls the next block's first
    attnv); reciprocal_approx_fast over both denominator rows -> dinv2;
    gpsimd broadcast into rb [128,512]; one tensor_mul per head writes the
    normalized bf16 uT. Only the FINAL block's norm reads PSUM directly
    (nothing recycles psU after it; shortens the end-of-run chain).
  - proj: lhsT = uT tile, rhs = w_proj; + b_proj via broadcast add.
    Output lands token-major [T, 768] == final layout.
  - scores are emitted one key-tile ahead of attn@V so exp latency is
    hidden; QKV of the next batch / proj of the previous batch are
    interleaved into the attention blocks as TensorE filler.

Schedule refinements vs the first working version (each A/B-measured
in-process against the previous best — cross-process exec times vary
+/-30us with the host window, in-process samples are +/-1.5us):
  - bias DMAs are emitted FIRST (DMA waits have running-threshold
    semantics: emitted last they paced the first evacuations, and with them
    the PE, behind the whole x/wqkv stream at startup).
  - the prologue computes only Q/K pairs 0,1, as two 4-unit waves with the
    k-loop OUTERMOST across 4 PSUM accumulators (2 psQ bufs + both halves
    of a psS tile), so the PE consumes each x/wqkv chunk as its DMA lands
    instead of one unit serially riding the chunk arrivals; only the wqkv
    columns those waves touch ([0:1024], contiguous 2KB runs — finer
    column slices degrade to 256-512B DMA runs and starve later fills)
    ride the startup-critical stream. V nh=0 units run as the first
    attention block's filler (popped one key-tile ahead of its attnv),
    which starts the exp pipeline ~15us earlier.
  - the deferred norm is emitted at jt==1 AFTER the filler pop: its muls
    wait on the cross-engine recip->broadcast chain, and DVE executes in
    order — emitted before the pop they delay the fill's PSUM evacuation
    and stall the PE ~1.4us per block.
  - b1's ih=0 projection tiles (t 0-3) join the filler at the last
    attention block (their uT columns complete at its jt==1 norm); the
    reserved filler is drained BEFORE the final norm (Tile's tile-granular
    dependency would otherwise chain those independent proj tiles behind
    it, a measured 5.3us PE gap), covering the final norm chain.
"""

import contextlib
import ctypes
import os
import sys
import types

import numpy as np

# ---------------------------------------------------------------------------
# NTFF profiling shim: bass_utils's trace path imports
# antenv.axon_hooks.get_axon_ntff_profile_hook, which this container's antenv
# lacks. Register a ctypes-based equivalent so BASS_TRACE=1 works. Harmless
# if tracing is never requested.
# ---------------------------------------------------------------------------


def _install_ntff_shim():
    if "antenv.axon_hooks" in sys.modules:
        return
    so_path = "/opt/axon/libaxon_pjrt.so"
    hook = None
    try:
        lib = ctypes.CDLL(so_path)
        if hasattr(lib, "axon_start_nrt_profile"):
            lib.axon_start_nrt_profile.argtypes = [
                ctypes.POINTER(ctypes.c_int64),
                ctypes.c_size_t,
            ]
            lib.axon_start_nrt_profile.restype = ctypes.c_int64
            lib.axon_stop_nrt_profile.argtypes = [ctypes.c_char_p]
            lib.axon_stop_nrt_profile.restype = ctypes.c_int64

            @contextlib.contextmanager
            def _hook(output_dir, device_ids):
                import jax

                jax.devices()
                if device_ids:
                    ids = (ctypes.c_int64 * len(device_ids))(*device_ids)
                    rc = lib.axon_start_nrt_profile(ids, len(device_ids))
                else:
                    rc = lib.axon_start_nrt_profile(None, 0)
                if rc != 0:
                    raise RuntimeError(f"axon_start_nrt_profile rc={rc}")
                try:
                    yield
                finally:
                    n = lib.axon_stop_nrt_profile(str(output_dir).encode())
                    print(f"ntff profile: {n} file(s) in {output_dir}",
                          file=sys.stderr)

            hook = _hook
    except OSError:
        pass
    mod = types.ModuleType("antenv.axon_hooks")
    mod.get_axon_ntff_profile_hook = lambda: hook
    mod.set_axon_ntff_profile_hook = lambda h: None
    sys.modules["antenv.axon_hooks"] = mod


_install_ntff_shim()

import concourse.bacc as bacc  # noqa: E402
import concourse.mybir as mybir  # noqa: E402
import concourse.tile as tile  # noqa: E402
from concourse.bass_utils import run_bass_kernel_spmd  # noqa: E402

F32 = mybir.dt.float32
F32R = mybir.dt.float32r
BF16 = mybir.dt.bfloat16
AF = mybir.ActivationFunctionType

# Problem constants (per core)
NB = 2        # batches per core
TN = 1024     # tokens per batch
T = NB * TN   # tokens per core
D = 768
H = 12
HD = 64
D3 = 3 * D
KT = D // 128          # 6 contraction tiles
NPAIR = H // 2         # 6 head pairs
NJT = TN // 128        # 8 key tiles per batch
SCALE = HD ** -0.5


def build():
    nc = bacc.Bacc(None)
    xT_d = nc.declare_dram_parameter("xT", [D, T], BF16, isOutput=False)
    wqkv_d = nc.declare_dram_parameter("wqkv", [D, D3], BF16, isOutput=False)
    wproj_d = nc.declare_dram_parameter("wproj", [D, D], BF16, isOutput=False)
    bqk_d = nc.declare_dram_parameter("bqk", [128, 12], F32, isOutput=False)
    bv_d = nc.declare_dram_parameter("bv", [1, D], BF16, isOutput=False)
    bproj_d = nc.declare_dram_parameter("bproj", [1, D], BF16, isOutput=False)
    out_d = nc.declare_dram_parameter("out", [T, D], F32, isOutput=True)

    with tile.TileContext(nc) as tc:
        with (
            nc.allow_low_precision(reason="bf16 attention pipeline"),
            tc.tile_pool(name="const", bufs=1) as cpool,
            tc.tile_pool(name="xu", bufs=2) as xupool,
            tc.tile_pool(name="qk", bufs=2) as qkpool,
            tc.tile_pool(name="vsb", bufs=2) as vpool,
            tc.tile_pool(name="esb", bufs=5) as epool,
            tc.tile_pool(name="stg", bufs=3) as spool,
            tc.tile_pool(name="dnv", bufs=3) as dpool,
            tc.tile_pool(name="rbp", bufs=4) as rbpool,
            tc.tile_pool(name="osb", bufs=3) as opool,
            tc.tile_pool(name="psS", bufs=2, space="PSUM") as psS,
            tc.tile_pool(name="psU", bufs=2, space="PSUM") as psU,
            tc.tile_pool(name="psQ", bufs=2, space="PSUM") as psQ,
        ):
            def dma_striped(dst, src, stripes):
                # split one logical transfer into partition stripes — each
                # dma_start lands on its own hardware queue, so this divides
                # the per-queue descriptor serialization
                step = 128 // stripes
                for s in range(stripes):
                    nc.sync.dma_start(
                        dst[s * step : (s + 1) * step], src[s * step : (s + 1) * step]
                    )

            wqkv = cpool.tile([128, KT, D3], BF16, tag="wqkv")
            wqkv_src = wqkv_d.ap().rearrange("(ko p) n -> p ko n", p=128)

            # --- per-batch contexts -------------------------------------
            class Batch:
                pass

            bats = []
            for b in range(NB):
                B_ = Batch()
                B_.tok0 = b * TN
                B_.qT = qkpool.tile([128, NPAIR, TN], BF16, tag="qT",
                                    name=f"qT{b}")
                B_.kT = qkpool.tile([128, NPAIR, TN], BF16, tag="kT",
                                    name=f"kT{b}")
                B_.vsb = vpool.tile([128, NJT, H, HD + 1], BF16, tag="v",
                                    name=f"v{b}")
                B_.uT = xupool.tile([128, KT, TN], BF16, tag="u",
                                    name=f"uT{b}")
                bats.append(B_)
            # both batches share one x tile: full 2048-token rows give 4KB
            # DMA runs (half the descriptor count of per-batch chunks)
            xcore = xupool.tile([128, KT, T], BF16, tag="x", bufs=1)

            xT_src = xT_d.ap().rearrange("(ko p) n -> p ko n", p=128)

            def wq_cols(k, c0, c1):
                nc.sync.dma_start(
                    wqkv[:, k : k + 1, c0:c1], wqkv_src[:, k : k + 1, c0:c1]
                )

            def emit_qk_unit(B_, m, ih):
                # one [128 feat, 512 tok] Q^T or K^T tile (m<6: Q, else K)
                dst = B_.qT if m < 6 else B_.kT
                hp = m % 6
                ps = psQ.tile([128, 512], F32, tag="ps")
                i0 = B_.tok0 + ih * 512
                for k in range(KT):
                    nc.tensor.matmul(
                        ps[:],
                        wqkv[:, k, m * 128 : (m + 1) * 128],
                        xcore[:, k, i0 : i0 + 512],
                        start=(k == 0),
                        stop=(k == KT - 1),
                    )
                nc.vector.tensor_scalar_add(
                    dst[:, hp, ih * 512 : (ih + 1) * 512],
                    ps[:],
                    bqk[:, m : m + 1],
                )

            def emit_v_unit(B_, t, nh):
                # one [128 tok, 384 feat] V tile into the v_ext slots
                ps = psQ.tile([128, 384], F32, tag="ps")
                t0_ = B_.tok0 + t * 128
                for k in range(KT):
                    nc.tensor.matmul(
                        ps[:],
                        xcore[:, k, t0_ : t0_ + 128],
                        wqkv[:, k, 2 * D + nh * 384 : 2 * D + (nh + 1) * 384],
                        start=(k == 0),
                        stop=(k == KT - 1),
                    )
                nc.vector.tensor_add(
                    B_.vsb[:, t, nh * 6 : (nh + 1) * 6, 0:HD],
                    ps[:],
                    bvb[:, nh * 384 : (nh + 1) * 384],
                )

            def emit_qk_wave(B_, units):
                """Up to 4 QK units accumulated k-chunk-interleaved so the PE
                consumes each x/wqkv chunk as its DMA lands (the serial unit
                order stalls ~2.6us per chunk at startup). Accumulators: the
                two psQ bufs plus both halves of one psS tile (idle during
                the prologue)."""
                accs = []
                wide = psS.tile([128, 1024], F32, tag="s")
                for i in range(len(units)):
                    if i < 2:
                        accs.append(
                            psQ.tile([128, 512], F32, tag="ps",
                                     name=f"pwq{i}")[:]
                        )
                    else:
                        accs.append(wide[:, (i - 2) * 512 : (i - 1) * 512])
                for k in range(KT):
                    for acc, (m, ih) in zip(accs, units):
                        i0 = B_.tok0 + ih * 512
                        nc.tensor.matmul(
                            acc,
                            wqkv[:, k, m * 128 : (m + 1) * 128],
                            xcore[:, k, i0 : i0 + 512],
                            start=(k == 0),
                            stop=(k == KT - 1),
                        )
                for acc, (m, ih) in zip(accs, units):
                    dst = B_.qT if m < 6 else B_.kT
                    nc.vector.tensor_scalar_add(
                        dst[:, m % 6, ih * 512 : (ih + 1) * 512],
                        acc,
                        bqk[:, m : m + 1],
                    )

            def emit_v_wave(B_, ts, nh):
                """Up to 4 V units, k-chunk-interleaved (see emit_qk_wave)."""
                accs = []
                wide = psS.tile([128, 1024], F32, tag="s")
                for i in range(len(ts)):
                    if i < 2:
                        accs.append(
                            psQ.tile([128, 384], F32, tag="ps",
                                     name=f"pwv{i}")[:]
                        )
                    else:
                        accs.append(wide[:, (i - 2) * 512 : (i - 2) * 512 + 384])
                for k in range(KT):
                    for acc, t in zip(accs, ts):
                        t0_ = B_.tok0 + t * 128
                        nc.tensor.matmul(
                            acc,
                            xcore[:, k, t0_ : t0_ + 128],
                            wqkv[:, k, 2 * D + nh * 384 : 2 * D + (nh + 1) * 384],
                            start=(k == 0),
                            stop=(k == KT - 1),
                        )
                for acc, t in zip(accs, ts):
                    nc.vector.tensor_add(
                        B_.vsb[:, t, nh * 6 : (nh + 1) * 6, 0:HD],
                        acc,
                        bvb[:, nh * 384 : (nh + 1) * 384],
                    )

            def emit_proj_unit(B_, t, nh):
                ps = psQ.tile([128, 384], F32, tag="ps")
                for k in range(KT):
                    nc.tensor.matmul(
                        ps[:],
                        B_.uT[:, k, t * 128 : (t + 1) * 128],
                        wproj[:, k, nh * 384 : (nh + 1) * 384],
                        start=(k == 0),
                        stop=(k == KT - 1),
                    )
                ot = opool.tile([128, 384], F32, tag="o")
                nc.vector.tensor_add(
                    ot[:], ps[:], bprojb[:, nh * 384 : (nh + 1) * 384]
                )
                nc.sync.dma_start(
                    out_d.ap()[
                        B_.tok0 + t * 128 : B_.tok0 + (t + 1) * 128,
                        nh * 384 : (nh + 1) * 384,
                    ],
                    ot[:],
                )

            # ---- filler queue: TensorE work interleaved into attention ---
            fill_queue = []

            def pop_fill(n, reserve=0):
                for _ in range(n):
                    if len(fill_queue) > reserve:
                        fill_queue.pop(0)()

            def emit_norm(pending):
                """Broadcast 1/denom (GpSimd, otherwise idle; full-tile dst
                from partition 0 — the only pattern the library op supports)
                and write the normalized pair block into uT. Reads the
                SBUF-staged U (staging decouples the psU buffer recycling:
                consecutive blocks share psU slots, and a norm reading PSUM
                directly stalls the next block's first attnv ~0.6us). The
                FINAL block skips staging and reads PSUM directly (nothing
                recycles psU afterward; shortens the end-of-run chain)."""
                B_, hp, ih, ust, dinv2, from_psum = pending
                for h in range(2):
                    rb = rbpool.tile([128, 512], F32, tag="rb",
                                     name=f"rb{h}")
                    nc.gpsimd.partition_broadcast(
                        rb[:], dinv2[0:1, h * 512 : (h + 1) * 512]
                    )
                    if from_psum:
                        # inputs partition-aligned at 0..63; write shifted
                        nc.vector.tensor_mul(
                            B_.uT[
                                h * 64 : (h + 1) * 64,
                                hp,
                                ih * 512 : (ih + 1) * 512,
                            ],
                            ust[h][0:HD, :],
                            rb[0:HD, :],
                        )
                    else:
                        nc.vector.tensor_mul(
                            B_.uT[
                                h * 64 : (h + 1) * 64,
                                hp,
                                ih * 512 : (ih + 1) * 512,
                            ],
                            ust[h * 64 : (h + 1) * 64, :],
                            rb[h * 64 : (h + 1) * 64, :],
                        )

            def emit_attn_block(B_, hp, ih, pending, reserve=0,
                                pops=(0, 2, 4, 6), final=False):
                """One (head-pair, query-half) attention block: 8 key tiles of
                scores+exp+attnV, scores one jt ahead of attnV. Fillers pop
                BEFORE the attnv of that jt so a V fill popped at jt is
                emitted ahead of the attnv that consumes it."""
                i0 = ih * 512
                pu = [
                    psU.tile([HD + 1, 512], F32, tag="pu", name=f"pu{h}")
                    for h in range(2)
                ]
                prev_e = None

                def attnv(e, jt):
                    for h in range(2):
                        nc.tensor.matmul(
                            pu[h][:],
                            B_.vsb[:, jt, 2 * hp + h, :],
                            e[:, h * 512 : (h + 1) * 512],
                            start=(jt == 0),
                            stop=(jt == NJT - 1),
                        )

                for jt in range(NJT):
                    ps = psS.tile([128, 1024], F32, tag="s")
                    for h in range(2):
                        nc.tensor.matmul(
                            ps[:, h * 512 : (h + 1) * 512],
                            B_.kT[
                                h * 64 : (h + 1) * 64,
                                hp,
                                jt * 128 : (jt + 1) * 128,
                            ],
                            B_.qT[h * 64 : (h + 1) * 64, hp, i0 : i0 + 512],
                        )
                    e = epool.tile([128, 1024], BF16, tag="e")
                    nc.scalar.activation(e[:], ps[:], AF.Exp, scale=SCALE)
                    if prev_e is not None:
                        attnv(prev_e, jt - 1)
                    if jt == 1 and pending is not None:
                        # the deferred norm goes AFTER the jt==0 filler pop:
                        # its muls wait the cross-engine recip->broadcast
                        # chain, and DVE runs in order — emitted at jt==0
                        # they delay the fill's PSUM evacuation and stall
                        # the PE ~1.4us per block
                        emit_norm(pending)
                        pending = None
                    if jt in pops:
                        pop_fill(1, reserve)
                    prev_e = e
                attnv(prev_e, NJT - 1)
                if pending is not None:
                    emit_norm(pending)

                # evacuate: denominator rows to SBUF staging, then one
                # reciprocal over both; data rows -> ust staging. pu slots
                # release after the copies.
                dtmp = dpool.tile([1, 1024], F32, tag="dtmp", bufs=2)
                dinv2 = dpool.tile([1, 1024], F32, tag="dinv", bufs=2)
                # denominator extraction + reciprocal first: they head the
                # critical chain (recip -> broadcast -> normalize multiply)
                for h in range(2):
                    nc.vector.tensor_copy(
                        dtmp[0:1, h * 512 : (h + 1) * 512],
                        pu[h][HD : HD + 1, :],
                    )
                nc.vector.reciprocal_approx_fast(dinv2[0:1, :], dtmp[0:1, :])
                if final:
                    return (B_, hp, ih, pu, dinv2, True)
                ust = spool.tile([128, 512], F32, tag="ust")
                for h in range(2):
                    nc.vector.tensor_copy(
                        ust[h * 64 : (h + 1) * 64, :], pu[h][0:HD, :]
                    )
                return (B_, hp, ih, ust, dinv2, False)

            # ================= global schedule =================
            b0, b1 = bats
            # biases FIRST: DMA waits are running-threshold (waiting on
            # transfer N waits on all earlier ones), and the first QK/V
            # evacuations need these — emitted late they pace the whole
            # prologue behind the x/wqkv bulk.
            bqk = cpool.tile([128, 12], F32, tag="bqk")
            nc.sync.dma_start(bqk[:], bqk_d.ap())
            bv1 = cpool.tile([1, D], BF16, tag="bv1")
            nc.sync.dma_start(bv1[:], bv_d.ap())
            bproj1 = cpool.tile([1, D], BF16, tag="bproj1")
            nc.sync.dma_start(bproj1[:], bproj_d.ap())

            # b0 x + wqkv interleaved per k chunk. wqkv moves as FULL k rows:
            # column-group slices would give 256B DMA runs (4x descriptor
            # overhead, measured to starve mid-attention fills) vs the 4.6KB
            # full-row runs.
            # per chunk, only the columns the QK waves touch ride the
            # critical stream: [0:1024] = all Q + K pairs 0,1 (contiguous
            # 2KB runs). K pairs 2-5 + V columns follow right after — V is
            # first consumed by block 0's attnv fills, a few us later.
            for k in range(KT):
                dma_striped(
                    xcore[:, k : k + 1, 0:TN], xT_src[:, k : k + 1, 0:TN], 2
                )
                dma_striped(
                    wqkv[:, k : k + 1, 0:1024], wqkv_src[:, k : k + 1, 0:1024], 2
                )
            for k in range(KT):
                dma_striped(
                    wqkv[:, k : k + 1, 1024:D3],
                    wqkv_src[:, k : k + 1, 1024:D3],
                    2,
                )

            # ones column of v_ext via memset — as a DMA this fragments
            # into ~12k 2-byte descriptors and stalls startup ~20us
            nc.vector.memset(b0.vsb[:, :, :, HD : HD + 1], 1.0)
            nc.vector.memset(b1.vsb[:, :, :, HD : HD + 1], 1.0)
            bvb = cpool.tile([128, D], BF16, tag="bvb")
            nc.gpsimd.partition_broadcast(bvb[:], bv1[:])
            bprojb = cpool.tile([128, D], BF16, tag="bprojb")
            nc.gpsimd.partition_broadcast(bprojb[:], bproj1[:])

            # b0 prologue, k-chunk-interleaved 4-unit waves: just Q/K pairs
            # 0,1 — attention starts right after; V nh=0 joins as the first
            # fills, consumed one key-tile ahead of block 0's attnv.
            emit_qk_wave(b0, [(0, 0), (6, 0), (0, 1), (6, 1)])
            emit_qk_wave(b0, [(1, 0), (7, 0), (1, 1), (7, 1)])

            # b1's x, then wproj — in fill consumption order.
            for k in range(KT):
                dma_striped(
                    xcore[:, k : k + 1, TN:T], xT_src[:, k : k + 1, TN:T], 2
                )
            wproj = cpool.tile([128, KT, D], BF16, tag="wproj")
            wproj_src = wproj_d.ap().rearrange("(ko p) n -> p ko n", p=128)
            for k in range(KT):
                nc.sync.dma_start(
                    wproj[:, k : k + 1, :], wproj_src[:, k : k + 1, :]
                )

            # filler during b0 attention, ordered by consumption deadline:
            # V nh=0 tile t by block 0's attnv(t), qk(hp) by block 2*hp,
            # V nh=1 by block 6 (pair 3).
            for t in range(NJT):
                fill_queue.append(lambda t=t: emit_v_unit(b0, t, 0))
            for hp in (2, 3):
                for m in (hp, hp + 6):
                    for ih in range(2):
                        fill_queue.append(
                            lambda m=m, ih=ih: emit_qk_unit(b0, m, ih)
                        )
            for t in range(NJT):
                fill_queue.append(
                    lambda t=t: emit_v_unit(b0, t, 1)
                )
            for hp in (4, 5):
                for m in (hp, hp + 6):
                    for ih in range(2):
                        fill_queue.append(
                            lambda m=m, ih=ih: emit_qk_unit(b0, m, ih)
                        )
            for m in (0, 6, 1, 7):
                for ih in range(2):
                    fill_queue.append(
                        lambda m=m, ih=ih: emit_qk_unit(b1, m, ih)
                    )
            for t in range(NJT):
                fill_queue.append(lambda t=t: emit_v_unit(b1, t, 0))

            pending = None
            for hp in range(NPAIR):
                for ih in range(2):
                    # block 0 pops every jt: it consumes the 8 V nh=0 fills
                    # just ahead of its own attnv
                    pops = tuple(range(NJT)) if (hp, ih) == (0, 0) else (0, 2, 4, 6)
                    pending = emit_attn_block(b0, hp, ih, pending, pops=pops)

            # drain b0 leftovers, then queue b1's remaining QKV (deadline
            # order; pair hp is needed by block 2*hp) and b0's proj as
            # filler during b1 attention
            while fill_queue:
                fill_queue.pop(0)()
            for hp in (2, 3):
                for m in (hp, hp + 6):
                    for ih in range(2):
                        fill_queue.append(
                            lambda m=m, ih=ih: emit_qk_unit(b1, m, ih)
                        )
            for t in range(NJT):
                fill_queue.append(lambda t=t: emit_v_unit(b1, t, 1))
            for hp in (4, 5):
                for m in (hp, hp + 6):
                    for ih in range(2):
                        fill_queue.append(
                            lambda m=m, ih=ih: emit_qk_unit(b1, m, ih)
                        )
            for t in range(NJT):
                for nh in range(2):
                    fill_queue.append(
                        lambda t=t, nh=nh: emit_proj_unit(b0, t, nh)
                    )

            # b1 attention (hp-major, like b0). Before the LAST block, b1's
            # ih=0 projection tiles (t 0-3, complete once the (5,0) norm
            # runs at that block's jt==0) join the filler; reserve holds a
            # few back to cover the final norm chain.
            for hp in range(NPAIR):
                for ih in range(2):
                    last = (hp, ih) == (NPAIR - 1, 1)
                    if last:
                        for t in range(NJT // 2):
                            for nh in range(2):
                                fill_queue.append(
                                    lambda t=t, nh=nh: emit_proj_unit(
                                        b1, t, nh
                                    )
                                )
                    # the last block's first pop waits until jt==1, after
                    # the (5,0) norm lands the uT columns its b1-proj
                    # fills read
                    pending = emit_attn_block(
                        b1, hp, ih, pending,
                        reserve=7 if last else 0,
                        pops=(1, 2, 4, 6) if last else (0, 2, 4, 6),
                        final=last,
                    )
            # drain the reserved filler BEFORE the final norm: the drained
            # proj tiles only read ih=0 uT columns, but Tile's tile-granular
            # dependency would chain them behind the final norm if emitted
            # after it (measured 5.3us PE gap). Then the final norm and the
            # ih=1 proj tiles that genuinely need it.
            while fill_queue:
                fill_queue.pop(0)()
            emit_norm(pending)
            for t in range(NJT // 2, NJT):
                for nh in range(2):
                    emit_proj_unit(b1, t, nh)

    nc.compile()
    return nc


_NC_CACHE = None


def _get_nc():
    global _NC_CACHE
    if _NC_CACHE is None:
        _NC_CACHE = build()
    return _NC_CACHE


def _prep_core_inputs(x_c, w_qkv, b_qkv, w_proj, b_proj):
    """Host-side layout prep for one core. x_c: [2, 1024, 768]."""
    xT = np.ascontiguousarray(x_c.reshape(T, D).T).astype(np.float32)
    bqk = np.ascontiguousarray(b_qkv[: 12 * 128].reshape(12, 128).T)
    import ml_dtypes

    bf = ml_dtypes.bfloat16
    return {
        "xT": np.ascontiguousarray(xT.astype(bf)),
        "wqkv": np.ascontiguousarray(w_qkv.astype(bf)),
        "wproj": np.ascontiguousarray(w_proj.astype(bf)),
        "bqk": bqk.astype(np.float32),
        "bv": np.ascontiguousarray(b_qkv[2 * D :].reshape(1, D).astype(bf)),
        "bproj": np.ascontiguousarray(b_proj.reshape(1, D).astype(bf)),
    }


def kernel(x, w_qkv, b_qkv, w_proj, b_proj):
    x = np.asarray(x, dtype=np.float32)
    w_qkv = np.asarray(w_qkv, dtype=np.float32)
    b_qkv = np.asarray(b_qkv, dtype=np.float32)
    w_proj = np.asarray(w_proj, dtype=np.float32)
    b_proj = np.asarray(b_proj, dtype=np.float32)
    B, N, Dd = x.shape
    assert (B, N, Dd) == (16, 1024, 768)

    nc = _get_nc()
    in_maps = [
        _prep_core_inputs(x[2 * c : 2 * c + 2], w_qkv, b_qkv, w_proj, b_proj)
        for c in range(8)
    ]
    res = run_bass_kernel_spmd(nc, in_maps, core_ids=list(range(8)))
    out = np.empty((B, N, Dd), dtype=np.float32)
    for c in range(8):
        out[2 * c : 2 * c + 2] = res.results[c]["out"].reshape(2, N, Dd)
    kernel.last_results = res
    return out
